# revision 37
# baseline (speedup 1.0000x reference)
"""AtlasMAG block: host glue + one Bass SPMD device kernel on 8 TRN2 NeuronCores.

Device offload (token-data-parallel: each core owns 512 tokens, full weights,
zero collectives):
  1. memory-branch MLP with symmetry-folded phi2 features
     (K: 64 + 64*65/2 = 2144 -> pad 2176 instead of 4160; exact math since
      z_i z_j and z_j z_i share one folded weight row)
  2. residual add (x + attn + mem), rmsnorm, gated FFN, final residual --
     all fused in the same NEFF launch.
Host (numpy): rmsnorm/qkv/rope/gamma/cummean/attention/TTL grads/Newton-Schulz.
"""
import sys
import types

sys.path.insert(0, "/opt/trn_rl_repo")


def _ensure_ntff_hook():
    """Restore the NTFF profiling hook if the image's antenv lacks it.

    trn_boot.boot() registers this hook at interpreter start only when
    antenv.axon_hooks is importable; on images without that module, tracing
    (and exec-time measurement) silently degrades. Recreate the module with
    the same ctypes-based hook trn_boot would have installed. No-op when the
    real module exists.
    """
    try:
        import antenv.axon_hooks  # noqa: F401
        return
    except ImportError:
        pass
    except Exception:
        return
    try:
        import antenv
        mod = types.ModuleType("antenv.axon_hooks")
        _hook = [None]

        def set_axon_ntff_profile_hook(hook):
            _hook[0] = hook

        def get_axon_ntff_profile_hook():
            if _hook[0] is None:
                try:
                    from trn_agent_boot.trn_boot import _ntff_profile_via_ctypes
                    _hook[0] = _ntff_profile_via_ctypes("/opt/axon/libaxon_pjrt.so")
                except Exception:
                    _hook[0] = None
            return _hook[0]

        mod.set_axon_ntff_profile_hook = set_axon_ntff_profile_hook
        mod.get_axon_ntff_profile_hook = get_axon_ntff_profile_hook
        sys.modules["antenv.axon_hooks"] = mod
        antenv.axon_hooks = mod
    except Exception:
        pass


_ensure_ntff_hook()

import numpy as np
import ml_dtypes

import concourse.bacc as bacc
import concourse.mybir as mybir
import concourse.tile as tile
from concourse.bass_utils import run_bass_kernel_spmd

BF16 = ml_dtypes.bfloat16
FP8 = ml_dtypes.float8_e4m3
N_CORES = 8
B, S, D, H = 2, 2048, 1024, 16
HD = D // H                      # 64
M_HID = 2 * D                    # 2048
FFN_H = int(D * 4 * 2 / 3)       # 2730 SwiGLU hidden
FFN_HP = 2816                    # 22 * 128
TOK = B * S                      # 4096
TT = TOK // N_CORES              # 512 tokens per core
EPS = 1e-6
OMEGA_W, OMEGA_DECAY = 64, 0.95
TTL_ALPHA, TTL_ETA, NS_ITERS = 0.999, 0.01, 5

# folded phi2 feature dim: 64 linear + 64*65/2 sym quad = 2144 -> pad
KF = 2144
KC_M = 18                        # mem L1 k-chunks (even, for fp8 DoubleRow)
K1 = KC_M * 128                  # 2304
MT_M = M_HID // 128              # 16 mem hidden tiles
OC = D // 128                    # 8 output-feature chunks
KC_F = D // 128                  # 8 ffn L1 k-chunks
MT_F = FFN_HP // 128             # 22 ffn hidden tiles

# fp8 (DoubleRow) per-branch switches; weights pre-scaled by powers of two so
# std-0.02 weights clear the e4m3 subnormal floor (2^-9), compensated on the
# psum-eviction path (activation scale= / fused DVE scalar ops).
FP8_MEM = True
FP8_FFN = True
WS_MEM1 = 64.0                   # w1fold scale
WS_MEM2 = 64.0                   # w2m scale
WS_FFN1 = 64.0                   # w13f scale
WS_FFN2 = 64.0                   # w2f scale
G2_SCALE = 8.0                   # g2 activation carry-scale

TRACE = False
EXEC_TIMES_NS = []

N_WARMUP = 12

_GRAPH_CACHE = {}

_IU_I, _IU_J = np.triu_indices(HD)


def _mm_group(nc, psum, lhsT3, rhs3, kc, dr):
    """Accumulate kc chunk-matmuls into psum; DoubleRow pairs when dr."""
    if dr:
        npair = kc // 2
        for i in range(npair):
            nc.tensor.matmul(
                psum, lhsT3[:, 2 * i:2 * i + 2, :], rhs3[:, 2 * i:2 * i + 2, :],
                start=(i == 0), stop=(i == npair - 1),
                perf_mode=mybir.MatmulPerfMode.DoubleRow)
    else:
        for i in range(kc):
            nc.tensor.matmul(
                psum, lhsT3[:, i:i + 1, :], rhs3[:, i:i + 1, :],
                start=(i == 0), stop=(i == kc - 1))


def _build_block_graph(fp8_mem, fp8_ffn):
    nc = bacc.Bacc("TRN2", target_bir_lowering=False, debug=False,
                   num_devices=N_CORES)
    bf = mybir.dt.bfloat16
    f32 = mybir.dt.float32
    fp8 = mybir.dt.float8e4
    dt_m = fp8 if fp8_mem else bf
    dt_f = fp8 if fp8_ffn else bf
    AF = mybir.ActivationFunctionType
    ALU = mybir.AluOpType

    feat = nc.declare_dram_parameter("feat", [128, KC_M, TT], dt_m, isOutput=False)
    w1m = nc.declare_dram_parameter("w1m", [128, MT_M * KC_M, 128], dt_m, isOutput=False)
    w2m = nc.declare_dram_parameter("w2m", [128, OC * MT_M, 128], dt_m, isOutput=False)
    w13f = nc.declare_dram_parameter("w13f", [128, MT_F * 2 * KC_F, 128], dt_f, isOutput=False)
    w2f = nc.declare_dram_parameter("w2f", [128, OC * MT_F, 128], dt_f, isOutput=False)
    rT = nc.declare_dram_parameter("rT", [128, OC, TT], f32, isOutput=False)
    out = nc.declare_dram_parameter("out", [128, OC, TT], f32, isOutput=True)

    inv_m1 = (1.0 / WS_MEM1) if fp8_mem else 1.0
    inv_m2 = (1.0 / WS_MEM2) if fp8_mem else 1.0
    inv_f1 = (1.0 / WS_FFN1) if fp8_ffn else 1.0
    g2s = G2_SCALE if fp8_ffn else 1.0
    inv_f2 = (1.0 / (WS_FFN2 * g2s)) if fp8_ffn else (1.0 / g2s)

    with tile.TileContext(nc) as tc:
        with tc.tile_pool(name="cst", bufs=1) as cst, \
             tc.tile_pool(name="big", bufs=1) as big, \
             tc.tile_pool(name="wstream", bufs=3) as ws, \
             tc.tile_pool(name="small", bufs=2) as sm, \
             tc.tile_pool(name="ps", bufs=7, space="PSUM") as ps:

            # -------- critical-first DMA: feat in graduated chunks split
            # across BOTH rings so the first mem-L1 pairs can start ASAP.
            feat_sb = big.tile([128, KC_M, TT], dt_m, tag="feat", name="feat_sb")
            nc.scalar.dma_start(feat_sb[:, 0:2, :], feat[:, 0:2, :])
            nc.sync.dma_start(feat_sb[:, 6:12, :], feat[:, 6:12, :])
            nc.scalar.dma_start(feat_sb[:, 2:6, :], feat[:, 2:6, :])
            nc.scalar.dma_start(feat_sb[:, 12:18, :], feat[:, 12:18, :])
            ones_f = cst.tile([128, 128], bf, tag="onef", name="ones_f")
            nc.vector.memset(ones_f[:], 1.0)

            # PE warm-up spanning the DMA ramp: dummy matmuls on a memset
            # scratch keep HAM at full clock until real data lands.
            scr = cst.tile([128, 256], bf, tag="scr", name="scr")
            nc.vector.memset(scr[:], 0.25)
            wps = ps.tile([128, 256], f32, tag="aux", bufs=1, name="wps")
            for i in range(N_WARMUP):
                nc.tensor.matmul(wps[:], scr[:, 0:128], scr[:],
                                 start=True, stop=True)

            eps_sb = cst.tile([128, 1], f32, tag="eps", name="eps_sb")
            nc.vector.memset(eps_sb[:], EPS)
            xmid = big.tile([128, OC, TT], f32, tag="xmid", name="xmid")
            h2 = big.tile([128, KC_F, TT], dt_f, tag="h2", name="h2")
            g_all = big.tile([128, MT_M, TT], dt_m, tag="g", name="g_all")
            g2_all = big.tile([128, MT_F, TT], dt_f, tag="g2", name="g2_all")

            # whole-tensor preloads (no WAR chains): rT and w2m ride the
            # scalar ring late in the mem-L1 loop (deferred so they don't
            # contend with the ramp-critical feat+w1m HBM burst); w2f goes
            # in the mem-L2 loop.
            w2m_sb = cst.tile([128, OC * MT_M, 128], dt_m, tag="w2m", name="w2m_sb")
            rt_sb = big.tile([128, OC, TT], f32, tag="rt", name="rt_sb")
            w2f_sb = cst.tile([128, OC * MT_F, 128], dt_f, tag="w2f", name="w2f_sb")
            deferred_dmas = {
                4: lambda: nc.scalar.dma_start(rt_sb[:, 0:4, :], rT[:, 0:4, :]),
                7: lambda: nc.scalar.dma_start(rt_sb[:, 4:8, :], rT[:, 4:8, :]),
                9: lambda: nc.scalar.dma_start(w2m_sb[:, 0:64, :], w2m[:, 0:64, :]),
                11: lambda: nc.scalar.dma_start(w2m_sb[:, 64:128, :], w2m[:, 64:128, :]),
            }

            # ---- memory MLP layer 1: g[m] = silu(w1m[m].T @ feat) ----
            # per-tile weight DMAs (294KB) on sync for a fine-grained ramp
            for m in range(MT_M):
                wm = ws.tile([128, KC_M, 128], dt_m, tag="w1m", bufs=8,
                             name=f"wm{m}")
                nc.sync.dma_start(wm[:], w1m[:, m * KC_M:(m + 1) * KC_M, :])
                pm = ps.tile([128, TT], f32, tag="mm", bufs=7, name=f"pm{m}")
                _mm_group(nc, pm[:], wm[:], feat_sb, KC_M, fp8_mem)
                if m in deferred_dmas:
                    deferred_dmas[m]()
                nc.scalar.activation(g_all[:, m:m + 1, :], pm[:], AF.Silu,
                                     scale=inv_m1)

            # ---- memory MLP layer 2 + residual; also squares for rmsnorm ----
            # The ssp sum-of-squares matmuls lag one iteration so the
            # in-order PE queue never waits on the sq eviction chain.
            # bcss accumulates per-token sum-of-squares via a [128,128] ones
            # stationary, so the result lands ALREADY BROADCAST across
            # partitions — no separate broadcast matmul needed.
            bcss = ps.tile([128, TT], f32, tag="aux", bufs=1, name="bcss")
            sqs = []
            pair_sums = []
            for o in range(OC):
                pm = ps.tile([128, TT], f32, tag="mm", bufs=7, name=f"po{o}")
                _mm_group(nc, pm[:], w2m_sb[:, o * MT_M:(o + 1) * MT_M, :],
                          g_all, MT_M, fp8_mem)
                if o == 1:
                    nc.scalar.dma_start(w2f_sb[:, 0:88, :], w2f[:, 0:88, :])
                elif o == 4:
                    nc.scalar.dma_start(w2f_sb[:, 88:176, :], w2f[:, 88:176, :])
                if o in (3, 5, 7):
                    # pre-summed square pairs -> only 4 bcss matmuls
                    k = (o - 3) // 2
                    nc.tensor.matmul(bcss[:], ones_f[:], pair_sums[k][:],
                                     start=(o == 3), stop=False)
                # xmid_o = pm/scale + rT_o
                nc.vector.scalar_tensor_tensor(
                    xmid[:, o:o + 1, :], pm[:], inv_m2, rt_sb[:, o:o + 1, :],
                    ALU.mult, ALU.add)
                sq = sm.tile([128, TT], bf, tag="sq", bufs=4, name=f"sq{o}")
                if o == OC - 1:
                    # last chunk: h2 copy first (ACT), square on DVE — both
                    # shorten the boundary critical chain
                    nc.scalar.copy(h2[:, o:o + 1, :], xmid[:, o:o + 1, :])
                    nc.vector.tensor_mul(sq[:], xmid[:, o:o + 1, :],
                                         xmid[:, o:o + 1, :])
                else:
                    nc.scalar.square(sq[:], xmid[:, o:o + 1, :])
                    # un-normalized FFN input (n2w is folded into w13f on
                    # host; the 1/rms factor is applied at psum eviction)
                    nc.scalar.copy(h2[:, o:o + 1, :], xmid[:, o:o + 1, :])
                sqs.append(sq)
                if o % 2 == 1:
                    # pair-sum the two fresh squares (DVE, slack in L2)
                    sq2 = sm.tile([128, TT], bf, tag="sq2", bufs=4,
                                  name=f"sq2_{o}")
                    nc.vector.tensor_add(sq2[:], sqs[o - 1][:], sqs[o][:])
                    pair_sums.append(sq2)

            # ---- FFN layer 1 + deferred rmsnorm reduction ----
            # g2[m] = silu(inv*w1f[m].T@h2) * (w3f[m].T@h2); the rms inv on
            # the w3 path commutes through ffn_w2 and is applied at the FFN2
            # eviction instead (saves one DVE op per tile).
            # The rmsnorm chain (last bcss matmul -> Ln -> Exp, both on the
            # already-broadcast [128,TT] sums; ln+exp share one ACT table
            # set) is threaded between the first FFN1 groups so the
            # in-order PE queue never idles on it.
            s_sb = sm.tile([128, TT], f32, tag="s", bufs=1, name="s_sb")
            bc_sb = sm.tile([128, TT], f32, tag="bcs", bufs=1, name="bc_sb")
            bcf_sb = sm.tile([128, TT], f32, tag="bcf", bufs=1, name="bcf_sb")

            def _ffn1_evict(m, pa1, pa3):
                t1 = sm.tile([128, TT], bf, tag="tm1", bufs=2, name=f"t1_{m}")
                nc.vector.tensor_mul(t1[:], pa1[:], bc_sb[:])
                sa = sm.tile([128, TT], bf, tag="sa", bufs=2, name=f"sa{m}")
                nc.scalar.activation(sa[:], t1[:], AF.Silu, scale=inv_f1)
                # g2 = (pa3 * g2s/ws) * sa   (carries g2s, lacks inv)
                nc.vector.scalar_tensor_tensor(
                    g2_all[:, m:m + 1, :], pa3[:], g2s * inv_f1, sa[:],
                    ALU.mult, ALU.mult)

            pending = []
            fblocks = [(a, min(a + 4, MT_F)) for a in range(0, MT_F, 4)]
            for fb, (ta, tb) in enumerate(fblocks):
                wf = ws.tile([128, 4 * 2 * KC_F, 128], dt_f, tag="w13f",
                             bufs=3, name=f"wf{fb}")
                nc.sync.dma_start(
                    wf[:, 0:(tb - ta) * 2 * KC_F, :],
                    w13f[:, ta * 2 * KC_F:tb * 2 * KC_F, :])
                for s in range(tb - ta):
                    m = ta + s
                    base = s * 2 * KC_F
                    pa1 = ps.tile([128, TT], f32, tag="mm", bufs=7,
                                  name=f"pa1_{m}")
                    _mm_group(nc, pa1[:], wf[:, base:base + KC_F, :], h2,
                              KC_F, fp8_ffn)
                    pa3 = ps.tile([128, TT], f32, tag="mm", bufs=7,
                                  name=f"pa3_{m}")
                    _mm_group(nc, pa3[:], wf[:, base + KC_F:base + 2 * KC_F, :],
                              h2, KC_F, fp8_ffn)
                    if m == 0:
                        # last sum-of-squares accum, then the rsqrt chain:
                        # bc = exp(-0.5*ln(ms+eps)) on the broadcast sums
                        nc.tensor.matmul(bcss[:], ones_f[:], pair_sums[3][:],
                                         start=False, stop=True)
                        nc.scalar.activation(s_sb[:], bcss[:], AF.Ln,
                                             bias=eps_sb[:], scale=1.0 / D)
                        nc.scalar.activation(bc_sb[:], s_sb[:], AF.Exp,
                                             scale=-0.5)
                        nc.vector.tensor_scalar_mul(bcf_sb[:], bc_sb[:],
                                                    inv_f2)
                    if m < 1:
                        # evictions need bc_sb; defer until the bc chain has
                        # been emitted so no read precedes its write
                        pending.append((m, pa1, pa3))
                        continue
                    if m == 1:
                        for args in pending:
                            _ffn1_evict(*args)
                        pending = []
                    _ffn1_evict(m, pa1, pa3)

            # ---- FFN layer 2 + final residual ----
            # out_o = pm * (inv*inv_f2) + xmid_o; last chunk split in halves
            # to shorten the end-of-kernel eviction+store tail.
            for o in range(OC):
                pm = ps.tile([128, TT], f32, tag="mm", bufs=7, name=f"pf{o}")
                _mm_group(nc, pm[:], w2f_sb[:, o * MT_F:(o + 1) * MT_F, :],
                          g2_all, MT_F, fp8_ffn)
                halves = ((0, TT),) if o < OC - 1 else ((0, TT // 2), (TT // 2, TT))
                for ca, cb in halves:
                    w = cb - ca
                    tmul = sm.tile([128, w], f32, tag=f"oct{w}", bufs=3,
                                   name=f"tm{o}_{ca}")
                    nc.vector.tensor_mul(tmul[:], pm[:, ca:cb],
                                         bcf_sb[:, ca:cb])
                    oc_t = sm.tile([128, w], f32, tag=f"oc{w}", bufs=3,
                                   name=f"oc{o}_{ca}")
                    nc.vector.tensor_add(oc_t[:], tmul[:],
                                         xmid[:, o:o + 1, ca:cb])
                    nc.sync.dma_start(out[:, o:o + 1, ca:cb], oc_t[:])
    nc.compile()
    return nc


def _chunk_major(a, nchunks):
    """[nchunks*128, C] -> [128, nchunks, C]"""
    c = a.shape[1]
    return np.ascontiguousarray(
        a.reshape(nchunks, 128, c).transpose(1, 0, 2))


def _w_block_major(w, kc, mt):
    """[kc*128, mt*128] -> [128, mt*kc, 128]
    (m-major blocks: block m holds all kc chunks of the 128 cols of tile m)"""
    return np.ascontiguousarray(
        w.reshape(kc, 128, mt, 128).transpose(1, 2, 0, 3).reshape(128, mt * kc, 128))


def _w_block_major_o(w, mt, oc):
    """[mt*128, oc*128] -> [128, oc*mt, 128]
    (o-major blocks for layer-2 weights)"""
    return np.ascontiguousarray(
        w.reshape(mt, 128, oc, 128).transpose(1, 2, 0, 3).reshape(128, oc * mt, 128))


def _fold_w1(w1u):
    """[64 + 64*64, M] -> [2176, M] symmetric-folded + scaled by 1/sqrt(HD)."""
    m = w1u.shape[1]
    q = HD + _IU_I * HD + _IU_J
    qt = HD + _IU_J * HD + _IU_I
    w1q = w1u[q, :] + w1u[qt, :]
    diag = _IU_I == _IU_J
    w1q[diag] = w1u[q[diag], :]
    w1q *= 1.0 / np.sqrt(np.float32(HD))
    out = np.zeros((K1, m), np.float32)
    out[:HD] = w1u[:HD]
    out[HD:HD + w1q.shape[0]] = w1q
    return out


def _fold_feat(z):
    """z [N, 64] -> folded feat [N, 2176] = [z, z_i*z_j (i<=j)], zero-pad."""
    n = z.shape[0]
    out = np.zeros((n, K1), np.float32)
    out[:, :HD] = z
    out[:, HD:HD + len(_IU_I)] = z[:, _IU_I] * z[:, _IU_J]
    return out


def _cast_dev(a, fp8):
    if fp8:
        return np.clip(a, -240.0, 240.0).astype(FP8)
    return a.astype(BF16)


def _run_device(feat_f, r, w1fold, w2ms, w1p, w3p, w2p, norm2):
    key = ("block", FP8_MEM, FP8_FFN)
    if key not in _GRAPH_CACHE:
        _GRAPH_CACHE[key] = _build_block_graph(FP8_MEM, FP8_FFN)
    nc = _GRAPH_CACHE[key]

    s_m1 = WS_MEM1 if FP8_MEM else 1.0
    s_m2 = WS_MEM2 if FP8_MEM else 1.0
    s_f1 = WS_FFN1 if FP8_FFN else 1.0
    s_f2 = WS_FFN2 if FP8_FFN else 1.0

    w1m_d = _cast_dev(_w_block_major(w1fold * s_m1, KC_M, MT_M), FP8_MEM)
    w2m_d = _cast_dev(_w_block_major_o(w2ms * s_m2, MT_M, OC), FP8_MEM)
    a13 = np.concatenate(
        [(w1p * s_f1).reshape(KC_F, 128, MT_F, 128).transpose(1, 2, 0, 3),
         (w3p * s_f1).reshape(KC_F, 128, MT_F, 128).transpose(1, 2, 0, 3)],
        axis=2)                                     # [128, MT_F, 2*KC_F, 128]
    w13f_d = _cast_dev(
        np.ascontiguousarray(a13.reshape(128, MT_F * 2 * KC_F, 128)), FP8_FFN)
    w2f_d = _cast_dev(_w_block_major_o(w2p * s_f2, MT_F, OC), FP8_FFN)
    in_maps = []
    for c in range(N_CORES):
        t0 = c * TT
        featc = np.ascontiguousarray(feat_f[t0:t0 + TT].T)      # [K1, TT]
        rc = np.ascontiguousarray(r[t0:t0 + TT].T)              # [D, TT]
        in_maps.append({
            "feat": _cast_dev(_chunk_major(featc, KC_M), FP8_MEM),
            "w1m": w1m_d,
            "w2m": w2m_d,
            "w13f": w13f_d,
            "w2f": w2f_d,
            "rT": _chunk_major(rc, OC).astype(np.float32),
        })

    res = run_bass_kernel_spmd(nc, in_maps, list(range(N_CORES)), trace=TRACE)
    if res.exec_time_ns is not None:
        EXEC_TIMES_NS.append(res.exec_time_ns)

    outs = []
    for c in range(N_CORES):
        o = np.asarray(res.results[c]["out"]).astype(np.float32)  # [128, OC, TT]
        o = o.reshape(128, OC, TT).transpose(1, 0, 2).reshape(D, TT)
        outs.append(o.T)                                          # [TT, D]
    return np.concatenate(outs, axis=0)                           # [TOK, D]


# ---------------- host math ----------------

def _rmsnorm(x, w):
    return x * (1.0 / np.sqrt(np.mean(x * x, -1, keepdims=True) + EPS)) * w


def _sigmoid(x):
    return 1.0 / (1.0 + np.exp(-x))


def _silu(x):
    return x * _sigmoid(x)


def _rope(q, k):
    half = HD // 2
    inv = 1.0 / (10000.0 ** (np.arange(half, dtype=np.float32) / half))
    fr = np.arange(S, dtype=np.float32)[:, None] * inv[None, :]
    cos, sin = np.cos(fr), np.sin(fr)

    def rot(x):
        x1, x2 = x[..., :half], x[..., half:]
        return np.concatenate([x1 * cos - x2 * sin, x1 * sin + x2 * cos], -1)

    return rot(q), rot(k)


def _phi2(z):
    outer = (z[..., :, None] * z[..., None, :]).reshape(*z.shape[:-1], HD * HD)
    return np.concatenate([z, outer / np.sqrt(np.float32(HD))], -1)


def _newton_schulz(G):
    a, b, c = 3.4445, -4.7750, 2.0315
    X = (G / (np.linalg.norm(G) + 1e-7)).astype(np.float32)
    tall = X.shape[0] > X.shape[1]
    X = X.T if tall else X
    for _ in range(NS_ITERS):
        A = X @ X.T
        X = a * X + (b * A + c * (A @ A)) @ X
    return X.T if tall else X


def kernel(x, norm1_w, norm2_w, qkv_w, q_norm_w, k_norm_w, gamma_w1, gamma_w2,
           mem_wk, mem_w1, mem_w2, memory_gate, wo_w, ffn_w1, ffn_w2, ffn_w3):
    x = np.asarray(x, np.float32)
    f32 = np.float32

    h = _rmsnorm(x, np.asarray(norm1_w, f32))
    qkv = h.reshape(TOK, D) @ np.asarray(qkv_w, f32)
    q, k, v = np.split(qkv.reshape(B, S, 3 * D), 3, axis=-1)

    def heads(t):
        return t.reshape(B, S, H, HD).transpose(0, 2, 1, 3)

    q, k, v = heads(q), heads(k), heads(v)
    q = _rmsnorm(q, np.asarray(q_norm_w, f32))
    k = _rmsnorm(k, np.asarray(k_norm_w, f32))
    q, k = _rope(q, k)

    gamma = _sigmoid(_silu(h @ np.asarray(gamma_w1, f32)) @ np.asarray(gamma_w2, f32))

    k_cummean = np.cumsum(k, axis=2) / np.arange(1, S + 1, dtype=f32)[None, None, :, None]
    g = gamma[:, None, :, :]
    q_mem = g * q + (1.0 - g) * k_cummean
    q_mem_flat = q_mem.transpose(0, 2, 1, 3).reshape(B, S, D)
    v_flat = v.transpose(0, 2, 1, 3).reshape(B, S, D)

    # ---- TTL grads (w_omega nonzero only on last OMEGA_W positions) ----
    mem_wk = np.asarray(mem_wk, f32)
    mem_w1 = np.asarray(mem_w1, f32)
    mem_w2 = np.asarray(mem_w2, f32)
    pos = np.arange(S)
    dpow = (np.float32(OMEGA_DECAY) ** (S - 1 - pos).astype(f32)).astype(f32)
    dpow = np.where(pos >= S - OMEGA_W, dpow, 0.0).astype(f32)
    w_omega = gamma[..., 0] * dpow                     # (B,S)
    denom = np.sum(w_omega) + 1e-8

    T0 = S - OMEGA_W
    qm_t = q_mem_flat[:, T0:]                          # (B,64,D)
    v_t = v_flat[:, T0:]
    z_t = qm_t @ mem_wk                                # (B,64,HD)
    ft = _phi2(z_t)                                    # (B,64,F_POLY)
    a1 = ft @ mem_w1
    sg = _sigmoid(a1)
    h1 = a1 * sg
    pred = h1 @ mem_w2
    diff = pred - v_t
    dpred = (2.0 / denom) * w_omega[:, T0:, None] * diff
    g2 = np.einsum('btm,btd->md', h1, dpred).astype(f32)
    da1 = (dpred @ mem_w2.T) * (sg * (1.0 + a1 * (1.0 - sg)))
    g1 = np.einsum('btf,btm->fm', ft, da1).astype(f32)
    dfeat = da1 @ mem_w1.T
    dz = dfeat[..., :HD].copy()
    dO = dfeat[..., HD:].reshape(B, OMEGA_W, HD, HD)
    dz += np.einsum('btij,btj->bti', dO + dO.transpose(0, 1, 3, 2),
                    z_t) / np.sqrt(np.float32(HD))
    gk = np.einsum('btd,bte->de', qm_t, dz).astype(f32)

    mem_wk_u = TTL_ALPHA * mem_wk - TTL_ETA * _newton_schulz(gk)
    mem_w1_u = TTL_ALPHA * mem_w1 - TTL_ETA * _newton_schulz(g1)
    mem_w2_u = TTL_ALPHA * mem_w2 - TTL_ETA * _newton_schulz(g2)

    # ---- attention on host ----
    scale = HD ** -0.5
    attn_out = np.empty((B, H, S, HD), f32)
    causal_bias = np.triu(np.full((S, S), -np.inf, f32), 1)
    for b in range(B):
        for hh in range(H):
            sc = (q[b, hh] @ k[b, hh].T) * scale + causal_bias
            sc -= sc.max(-1, keepdims=True)
            e = np.exp(sc)
            p = e / e.sum(-1, keepdims=True)
            attn_out[b, hh] = p @ v[b, hh]
    attn_out = attn_out.transpose(0, 2, 1, 3).reshape(B, S, D) @ np.asarray(wo_w, f32)

    # ---- device: mem MLP fwd + residuals + rmsnorm + FFN ----
    r = (x + attn_out).reshape(TOK, D).astype(f32)
    z_full = (q_mem_flat @ mem_wk_u).reshape(TOK, HD)
    feat_f = _fold_feat(z_full)
    w1fold = _fold_w1(mem_w1_u)
    w2ms = (mem_w2_u * _sigmoid(np.asarray(memory_gate, f32))).astype(f32)

    # fold norm2_w into the FFN input weights: w.T @ (xmid*inv*n2w) =
    # inv * ((w*n2w).T @ xmid), with inv applied at psum eviction on device
    n2 = np.asarray(norm2_w, f32)[:, None]
    w1p = np.zeros((D, FFN_HP), f32)
    w1p[:, :FFN_H] = np.asarray(ffn_w1, f32) * n2
    w3p = np.zeros((D, FFN_HP), f32)
    w3p[:, :FFN_H] = np.asarray(ffn_w3, f32) * n2
    w2p = np.zeros((FFN_HP, D), f32)
    w2p[:FFN_H, :] = np.asarray(ffn_w2, f32)

    out = _run_device(feat_f, r, w1fold, w2ms, w1p, w3p, w2p,
                      np.asarray(norm2_w, f32))
    return out.reshape(B, S, D).astype(np.float32)



# revision 46
# speedup vs baseline: 1.0248x; 1.0248x over previous
"""AtlasMAG block: host glue + one Bass SPMD device kernel on 8 TRN2 NeuronCores.

Device offload (token-data-parallel: each core owns 512 tokens, full weights,
zero collectives):
  1. memory-branch MLP with symmetry-folded phi2 features
     (K: 64 + 64*65/2 = 2144 -> pad 2176 instead of 4160; exact math since
      z_i z_j and z_j z_i share one folded weight row)
  2. residual add (x + attn + mem), rmsnorm, gated FFN, final residual --
     all fused in the same NEFF launch.
Host (numpy): rmsnorm/qkv/rope/gamma/cummean/attention/TTL grads/Newton-Schulz.
"""
import sys
import types

sys.path.insert(0, "/opt/trn_rl_repo")


def _ensure_ntff_hook():
    """Restore the NTFF profiling hook if the image's antenv lacks it.

    trn_boot.boot() registers this hook at interpreter start only when
    antenv.axon_hooks is importable; on images without that module, tracing
    (and exec-time measurement) silently degrades. Recreate the module with
    the same ctypes-based hook trn_boot would have installed. No-op when the
    real module exists.
    """
    try:
        import antenv.axon_hooks  # noqa: F401
        return
    except ImportError:
        pass
    except Exception:
        return
    try:
        import antenv
        mod = types.ModuleType("antenv.axon_hooks")
        _hook = [None]

        def set_axon_ntff_profile_hook(hook):
            _hook[0] = hook

        def get_axon_ntff_profile_hook():
            if _hook[0] is None:
                try:
                    from trn_agent_boot.trn_boot import _ntff_profile_via_ctypes
                    _hook[0] = _ntff_profile_via_ctypes("/opt/axon/libaxon_pjrt.so")
                except Exception:
                    _hook[0] = None
            return _hook[0]

        mod.set_axon_ntff_profile_hook = set_axon_ntff_profile_hook
        mod.get_axon_ntff_profile_hook = get_axon_ntff_profile_hook
        sys.modules["antenv.axon_hooks"] = mod
        antenv.axon_hooks = mod
    except Exception:
        pass


_ensure_ntff_hook()

import numpy as np
import ml_dtypes

import concourse.bacc as bacc
import concourse.mybir as mybir
import concourse.tile as tile
from concourse.bass_utils import run_bass_kernel_spmd

BF16 = ml_dtypes.bfloat16
FP8 = ml_dtypes.float8_e4m3
N_CORES = 8
B, S, D, H = 2, 2048, 1024, 16
HD = D // H                      # 64
M_HID = 2 * D                    # 2048
FFN_H = int(D * 4 * 2 / 3)       # 2730 SwiGLU hidden
FFN_HP = 2816                    # 22 * 128
TOK = B * S                      # 4096
TT = TOK // N_CORES              # 512 tokens per core
EPS = 1e-6
OMEGA_W, OMEGA_DECAY = 64, 0.95
TTL_ALPHA, TTL_ETA, NS_ITERS = 0.999, 0.01, 5

# folded phi2 feature dim: 64 linear + 64*65/2 sym quad = 2144 -> pad
KF = 2144
KC_M = 18                        # mem L1 k-chunks (even, for fp8 DoubleRow)
K1 = KC_M * 128                  # 2304
MT_M = M_HID // 128              # 16 mem hidden tiles
OC = D // 128                    # 8 output-feature chunks
KC_F = D // 128                  # 8 ffn L1 k-chunks
MT_F = FFN_HP // 128             # 22 ffn hidden tiles

# fp8 (DoubleRow) per-branch switches; weights pre-scaled by powers of two so
# std-0.02 weights clear the e4m3 subnormal floor (2^-9), compensated on the
# psum-eviction path (activation scale= / fused DVE scalar ops).
FP8_MEM = True
FP8_FFN = True
WS_MEM1 = 64.0                   # w1fold scale
WS_MEM2 = 64.0                   # w2m scale
WS_FFN1 = 64.0                   # w13f scale
WS_FFN2 = 64.0                   # w2f scale
G2_SCALE = 8.0                   # g2 activation carry-scale

TRACE = False
EXEC_TIMES_NS = []

N_WARMUP = 12

_GRAPH_CACHE = {}

_IU_I, _IU_J = np.triu_indices(HD)


def _mm_group(nc, psum, lhsT3, rhs3, kc, dr):
    """Accumulate kc chunk-matmuls into psum; DoubleRow pairs when dr."""
    if dr:
        npair = kc // 2
        for i in range(npair):
            nc.tensor.matmul(
                psum, lhsT3[:, 2 * i:2 * i + 2, :], rhs3[:, 2 * i:2 * i + 2, :],
                start=(i == 0), stop=(i == npair - 1),
                perf_mode=mybir.MatmulPerfMode.DoubleRow)
    else:
        for i in range(kc):
            nc.tensor.matmul(
                psum, lhsT3[:, i:i + 1, :], rhs3[:, i:i + 1, :],
                start=(i == 0), stop=(i == kc - 1))


def _build_block_graph(fp8_mem, fp8_ffn):
    nc = bacc.Bacc("TRN2", target_bir_lowering=False, debug=False,
                   num_devices=N_CORES)
    bf = mybir.dt.bfloat16
    f32 = mybir.dt.float32
    fp8 = mybir.dt.float8e4
    dt_m = fp8 if fp8_mem else bf
    dt_f = fp8 if fp8_ffn else bf
    AF = mybir.ActivationFunctionType
    ALU = mybir.AluOpType

    feat = nc.declare_dram_parameter("feat", [128, KC_M, TT], dt_m, isOutput=False)
    w1m = nc.declare_dram_parameter("w1m", [128, MT_M * KC_M, 128], dt_m, isOutput=False)
    w2m = nc.declare_dram_parameter("w2m", [128, OC * MT_M, 128], dt_m, isOutput=False)
    w13f = nc.declare_dram_parameter("w13f", [128, MT_F * 2 * KC_F, 128], dt_f, isOutput=False)
    w2f = nc.declare_dram_parameter("w2f", [128, OC * MT_F, 128], dt_f, isOutput=False)
    rT = nc.declare_dram_parameter("rT", [128, OC, TT], f32, isOutput=False)
    out = nc.declare_dram_parameter("out", [128, OC, TT], f32, isOutput=True)

    inv_m1 = (1.0 / WS_MEM1) if fp8_mem else 1.0
    inv_m2 = (1.0 / WS_MEM2) if fp8_mem else 1.0
    inv_f1 = (1.0 / WS_FFN1) if fp8_ffn else 1.0
    g2s = G2_SCALE if fp8_ffn else 1.0
    inv_f2 = (1.0 / (WS_FFN2 * g2s)) if fp8_ffn else (1.0 / g2s)

    with tile.TileContext(nc) as tc:
        with tc.tile_pool(name="cst", bufs=1) as cst, \
             tc.tile_pool(name="big", bufs=1) as big, \
             tc.tile_pool(name="wstream", bufs=3) as ws, \
             tc.tile_pool(name="small", bufs=2) as sm, \
             tc.tile_pool(name="ps", bufs=7, space="PSUM") as ps:

            # -------- critical-first DMA: feat in graduated chunks split
            # across BOTH rings so the first mem-L1 pairs can start ASAP.
            feat_sb = big.tile([128, KC_M, TT], dt_m, tag="feat", name="feat_sb")
            nc.scalar.dma_start(feat_sb[:, 0:2, :], feat[:, 0:2, :])
            nc.sync.dma_start(feat_sb[:, 6:12, :], feat[:, 6:12, :])
            nc.scalar.dma_start(feat_sb[:, 2:6, :], feat[:, 2:6, :])
            nc.scalar.dma_start(feat_sb[:, 12:18, :], feat[:, 12:18, :])
            ones_f = cst.tile([128, 2, 128], fp8 if fp8_mem else bf,
                              tag="onef", name="ones_f")
            nc.vector.memset(ones_f[:], 1.0)

            # PE warm-up spanning the DMA ramp: dummy matmuls on a memset
            # scratch keep HAM at full clock until real data lands.
            scr = cst.tile([128, 256], bf, tag="scr", name="scr")
            nc.vector.memset(scr[:], 0.25)
            wps = ps.tile([128, 256], f32, tag="aux", bufs=1, name="wps")
            for i in range(N_WARMUP):
                nc.tensor.matmul(wps[:], scr[:, 0:128], scr[:],
                                 start=True, stop=True)

            eps_sb = cst.tile([128, 1], f32, tag="eps", name="eps_sb")
            nc.vector.memset(eps_sb[:], EPS)
            xmid = big.tile([128, OC, TT], f32, tag="xmid", name="xmid")
            h2 = big.tile([128, KC_F, TT], dt_f, tag="h2", name="h2")
            g_all = big.tile([128, MT_M, TT], dt_m, tag="g", name="g_all")
            g2_all = big.tile([128, MT_F, TT], dt_f, tag="g2", name="g2_all")

            # whole-tensor preloads (no WAR chains): rT and w2m ride the
            # scalar ring late in the mem-L1 loop (deferred so they don't
            # contend with the ramp-critical feat+w1m HBM burst); w2f goes
            # in the mem-L2 loop.
            w2m_sb = cst.tile([128, OC * MT_M, 128], dt_m, tag="w2m", name="w2m_sb")
            rt_sb = big.tile([128, OC, TT], f32, tag="rt", name="rt_sb")
            w2f_sb = cst.tile([128, OC * MT_F, 128], dt_f, tag="w2f", name="w2f_sb")
            deferred_dmas = {
                4: lambda: nc.scalar.dma_start(rt_sb[:, 0:4, :], rT[:, 0:4, :]),
                7: lambda: nc.scalar.dma_start(rt_sb[:, 4:8, :], rT[:, 4:8, :]),
                9: lambda: nc.scalar.dma_start(w2m_sb[:, 0:64, :], w2m[:, 0:64, :]),
                11: lambda: nc.scalar.dma_start(w2m_sb[:, 64:128, :], w2m[:, 64:128, :]),
            }

            # ---- memory MLP layer 1: g[m] = silu(w1m[m].T @ feat) ----
            # per-tile weight DMAs (294KB) on sync for a fine-grained ramp
            for m in range(MT_M):
                wm = ws.tile([128, KC_M, 128], dt_m, tag="w1m", bufs=8,
                             name=f"wm{m}")
                nc.sync.dma_start(wm[:], w1m[:, m * KC_M:(m + 1) * KC_M, :])
                pm = ps.tile([128, TT], f32, tag="mm", bufs=7, name=f"pm{m}")
                _mm_group(nc, pm[:], wm[:], feat_sb, KC_M, fp8_mem)
                if m in deferred_dmas:
                    deferred_dmas[m]()
                nc.scalar.activation(g_all[:, m:m + 1, :], pm[:], AF.Silu,
                                     scale=inv_m1)

            # ---- memory MLP layer 2 + residual; also squares for rmsnorm ----
            # The ssp sum-of-squares matmuls lag one iteration so the
            # in-order PE queue never waits on the sq eviction chain.
            # bcss accumulates per-token sum-of-squares via a [128,128] ones
            # stationary, so the result lands ALREADY BROADCAST across
            # partitions — no separate broadcast matmul needed.
            bcss = ps.tile([128, TT], f32, tag="aux", bufs=1, name="bcss")
            sq_all = big.tile([128, OC, TT], fp8 if fp8_mem else bf,
                              tag="sqall", name="sq_all")
            for o in range(OC):
                pm = ps.tile([128, TT], f32, tag="mm", bufs=7, name=f"po{o}")
                _mm_group(nc, pm[:], w2m_sb[:, o * MT_M:(o + 1) * MT_M, :],
                          g_all, MT_M, fp8_mem)
                if o == 1:
                    nc.scalar.dma_start(w2f_sb[:, 0:88, :], w2f[:, 0:88, :])
                elif o == 4:
                    nc.scalar.dma_start(w2f_sb[:, 88:176, :], w2f[:, 88:176, :])
                if o in (3, 5, 7) and fp8_mem:
                    # fp8 squares allow DR pairs: 4 accum matmuls, not 8
                    # (0.1% ms error after the 1024-wide mean: negligible)
                    k = (o - 3) // 2
                    nc.tensor.matmul(
                        bcss[:], ones_f[:], sq_all[:, 2 * k:2 * k + 2, :],
                        start=(o == 3), stop=False,
                        perf_mode=mybir.MatmulPerfMode.DoubleRow)
                # xmid_o = pm/scale + rT_o
                nc.vector.scalar_tensor_tensor(
                    xmid[:, o:o + 1, :], pm[:], inv_m2, rt_sb[:, o:o + 1, :],
                    ALU.mult, ALU.add)
                if o == OC - 1:
                    # last chunk: h2 copy first (ACT), square on DVE — both
                    # shorten the boundary critical chain
                    nc.scalar.copy(h2[:, o:o + 1, :], xmid[:, o:o + 1, :])
                    nc.vector.tensor_mul(sq_all[:, o:o + 1, :],
                                         xmid[:, o:o + 1, :],
                                         xmid[:, o:o + 1, :])
                else:
                    nc.scalar.square(sq_all[:, o:o + 1, :],
                                     xmid[:, o:o + 1, :])
                    # un-normalized FFN input (n2w is folded into w13f on
                    # host; the 1/rms factor is applied at psum eviction)
                    nc.scalar.copy(h2[:, o:o + 1, :], xmid[:, o:o + 1, :])

            # ---- FFN layer 1 + deferred rmsnorm reduction ----
            # g2[m] = silu(inv*w1f[m].T@h2) * (w3f[m].T@h2); the rms inv on
            # the w3 path commutes through ffn_w2 and is applied at the FFN2
            # eviction instead (saves one DVE op per tile).
            # The rmsnorm chain (last bcss matmul -> Ln -> Exp, both on the
            # already-broadcast [128,TT] sums; ln+exp share one ACT table
            # set) is threaded between the first FFN1 groups so the
            # in-order PE queue never idles on it.
            s_sb = sm.tile([128, TT], f32, tag="s", bufs=1, name="s_sb")
            bc_sb = sm.tile([128, TT], f32, tag="bcs", bufs=1, name="bc_sb")
            bcf_sb = sm.tile([128, TT], f32, tag="bcf", bufs=1, name="bcf_sb")

            def _ffn1_evict(m, pa1, pa3):
                t1 = sm.tile([128, TT], bf, tag="tm1", bufs=2, name=f"t1_{m}")
                nc.vector.tensor_mul(t1[:], pa1[:], bc_sb[:])
                sa = sm.tile([128, TT], bf, tag="sa", bufs=2, name=f"sa{m}")
                nc.scalar.activation(sa[:], t1[:], AF.Silu, scale=inv_f1)
                # g2 = (pa3 * g2s/ws) * sa   (carries g2s, lacks inv)
                nc.vector.scalar_tensor_tensor(
                    g2_all[:, m:m + 1, :], pa3[:], g2s * inv_f1, sa[:],
                    ALU.mult, ALU.mult)

            pending = []
            fblocks = [(a, min(a + 4, MT_F)) for a in range(0, MT_F, 4)]
            for fb, (ta, tb) in enumerate(fblocks):
                wf = ws.tile([128, 4 * 2 * KC_F, 128], dt_f, tag="w13f",
                             bufs=3, name=f"wf{fb}")
                nc.sync.dma_start(
                    wf[:, 0:(tb - ta) * 2 * KC_F, :],
                    w13f[:, ta * 2 * KC_F:tb * 2 * KC_F, :])
                for s in range(tb - ta):
                    m = ta + s
                    base = s * 2 * KC_F
                    pa1 = ps.tile([128, TT], f32, tag="mm", bufs=7,
                                  name=f"pa1_{m}")
                    _mm_group(nc, pa1[:], wf[:, base:base + KC_F, :], h2,
                              KC_F, fp8_ffn)
                    pa3 = ps.tile([128, TT], f32, tag="mm", bufs=7,
                                  name=f"pa3_{m}")
                    _mm_group(nc, pa3[:], wf[:, base + KC_F:base + 2 * KC_F, :],
                              h2, KC_F, fp8_ffn)
                    if m == 0:
                        # last sum-of-squares accum, then the rsqrt chain:
                        # bc = exp(-0.5*ln(ms+eps)) on the broadcast sums
                        nc.tensor.matmul(
                            bcss[:], ones_f[:], sq_all[:, OC - 2:OC, :],
                            start=False, stop=True,
                            perf_mode=mybir.MatmulPerfMode.DoubleRow)
                        nc.scalar.activation(s_sb[:], bcss[:], AF.Ln,
                                             bias=eps_sb[:], scale=1.0 / D)
                        nc.scalar.activation(bc_sb[:], s_sb[:], AF.Exp,
                                             scale=-0.5)
                        nc.vector.tensor_scalar_mul(bcf_sb[:], bc_sb[:],
                                                    inv_f2)
                    if m < 1:
                        # evictions need bc_sb; defer until the bc chain has
                        # been emitted so no read precedes its write
                        pending.append((m, pa1, pa3))
                        continue
                    if m == 1:
                        for args in pending:
                            _ffn1_evict(*args)
                        pending = []
                    _ffn1_evict(m, pa1, pa3)

            # ---- FFN layer 2 + final residual ----
            # out_o = pm * (inv*inv_f2) + xmid_o; last chunk split in halves
            # to shorten the end-of-kernel eviction+store tail.
            for o in range(OC):
                pm = ps.tile([128, TT], f32, tag="mm", bufs=7, name=f"pf{o}")
                _mm_group(nc, pm[:], w2f_sb[:, o * MT_F:(o + 1) * MT_F, :],
                          g2_all, MT_F, fp8_ffn)
                halves = ((0, TT),) if o < OC - 1 else ((0, 256), (256, 448),
                                                       (448, 512))
                for ca, cb in halves:
                    w = cb - ca
                    tmul = sm.tile([128, w], f32, tag=f"oct{w}", bufs=3,
                                   name=f"tm{o}_{ca}")
                    nc.vector.tensor_mul(tmul[:], pm[:, ca:cb],
                                         bcf_sb[:, ca:cb])
                    oc_t = sm.tile([128, w], f32, tag=f"oc{w}", bufs=3,
                                   name=f"oc{o}_{ca}")
                    nc.vector.tensor_add(oc_t[:], tmul[:],
                                         xmid[:, o:o + 1, ca:cb])
                    nc.sync.dma_start(out[:, o:o + 1, ca:cb], oc_t[:])
    nc.compile()
    return nc


def _chunk_major(a, nchunks):
    """[nchunks*128, C] -> [128, nchunks, C]"""
    c = a.shape[1]
    return np.ascontiguousarray(
        a.reshape(nchunks, 128, c).transpose(1, 0, 2))


def _w_block_major(w, kc, mt):
    """[kc*128, mt*128] -> [128, mt*kc, 128]
    (m-major blocks: block m holds all kc chunks of the 128 cols of tile m)"""
    return np.ascontiguousarray(
        w.reshape(kc, 128, mt, 128).transpose(1, 2, 0, 3).reshape(128, mt * kc, 128))


def _w_block_major_o(w, mt, oc):
    """[mt*128, oc*128] -> [128, oc*mt, 128]
    (o-major blocks for layer-2 weights)"""
    return np.ascontiguousarray(
        w.reshape(mt, 128, oc, 128).transpose(1, 2, 0, 3).reshape(128, oc * mt, 128))


def _fold_w1(w1u):
    """[64 + 64*64, M] -> [2176, M] symmetric-folded + scaled by 1/sqrt(HD)."""
    m = w1u.shape[1]
    q = HD + _IU_I * HD + _IU_J
    qt = HD + _IU_J * HD + _IU_I
    w1q = w1u[q, :] + w1u[qt, :]
    diag = _IU_I == _IU_J
    w1q[diag] = w1u[q[diag], :]
    w1q *= 1.0 / np.sqrt(np.float32(HD))
    out = np.zeros((K1, m), np.float32)
    out[:HD] = w1u[:HD]
    out[HD:HD + w1q.shape[0]] = w1q
    return out


def _fold_feat(z):
    """z [N, 64] -> folded feat [N, 2176] = [z, z_i*z_j (i<=j)], zero-pad."""
    n = z.shape[0]
    out = np.zeros((n, K1), np.float32)
    out[:, :HD] = z
    out[:, HD:HD + len(_IU_I)] = z[:, _IU_I] * z[:, _IU_J]
    return out


def _cast_dev(a, fp8):
    if fp8:
        return np.clip(a, -240.0, 240.0).astype(FP8)
    return a.astype(BF16)


def _run_device(feat_f, r, w1fold, w2ms, w1p, w3p, w2p, norm2):
    key = ("block", FP8_MEM, FP8_FFN)
    if key not in _GRAPH_CACHE:
        _GRAPH_CACHE[key] = _build_block_graph(FP8_MEM, FP8_FFN)
    nc = _GRAPH_CACHE[key]

    s_m1 = WS_MEM1 if FP8_MEM else 1.0
    s_m2 = WS_MEM2 if FP8_MEM else 1.0
    s_f1 = WS_FFN1 if FP8_FFN else 1.0
    s_f2 = WS_FFN2 if FP8_FFN else 1.0

    w1m_d = _cast_dev(_w_block_major(w1fold * s_m1, KC_M, MT_M), FP8_MEM)
    w2m_d = _cast_dev(_w_block_major_o(w2ms * s_m2, MT_M, OC), FP8_MEM)
    a13 = np.concatenate(
        [(w1p * s_f1).reshape(KC_F, 128, MT_F, 128).transpose(1, 2, 0, 3),
         (w3p * s_f1).reshape(KC_F, 128, MT_F, 128).transpose(1, 2, 0, 3)],
        axis=2)                                     # [128, MT_F, 2*KC_F, 128]
    w13f_d = _cast_dev(
        np.ascontiguousarray(a13.reshape(128, MT_F * 2 * KC_F, 128)), FP8_FFN)
    w2f_d = _cast_dev(_w_block_major_o(w2p * s_f2, MT_F, OC), FP8_FFN)
    in_maps = []
    for c in range(N_CORES):
        t0 = c * TT
        featc = np.ascontiguousarray(feat_f[t0:t0 + TT].T)      # [K1, TT]
        rc = np.ascontiguousarray(r[t0:t0 + TT].T)              # [D, TT]
        in_maps.append({
            "feat": _cast_dev(_chunk_major(featc, KC_M), FP8_MEM),
            "w1m": w1m_d,
            "w2m": w2m_d,
            "w13f": w13f_d,
            "w2f": w2f_d,
            "rT": _chunk_major(rc, OC).astype(np.float32),
        })

    res = run_bass_kernel_spmd(nc, in_maps, list(range(N_CORES)), trace=TRACE)
    if res.exec_time_ns is not None:
        EXEC_TIMES_NS.append(res.exec_time_ns)

    outs = []
    for c in range(N_CORES):
        o = np.asarray(res.results[c]["out"]).astype(np.float32)  # [128, OC, TT]
        o = o.reshape(128, OC, TT).transpose(1, 0, 2).reshape(D, TT)
        outs.append(o.T)                                          # [TT, D]
    return np.concatenate(outs, axis=0)                           # [TOK, D]


# ---------------- host math ----------------

def _rmsnorm(x, w):
    return x * (1.0 / np.sqrt(np.mean(x * x, -1, keepdims=True) + EPS)) * w


def _sigmoid(x):
    return 1.0 / (1.0 + np.exp(-x))


def _silu(x):
    return x * _sigmoid(x)


def _rope(q, k):
    half = HD // 2
    inv = 1.0 / (10000.0 ** (np.arange(half, dtype=np.float32) / half))
    fr = np.arange(S, dtype=np.float32)[:, None] * inv[None, :]
    cos, sin = np.cos(fr), np.sin(fr)

    def rot(x):
        x1, x2 = x[..., :half], x[..., half:]
        return np.concatenate([x1 * cos - x2 * sin, x1 * sin + x2 * cos], -1)

    return rot(q), rot(k)


def _phi2(z):
    outer = (z[..., :, None] * z[..., None, :]).reshape(*z.shape[:-1], HD * HD)
    return np.concatenate([z, outer / np.sqrt(np.float32(HD))], -1)


def _newton_schulz(G):
    a, b, c = 3.4445, -4.7750, 2.0315
    X = (G / (np.linalg.norm(G) + 1e-7)).astype(np.float32)
    tall = X.shape[0] > X.shape[1]
    X = X.T if tall else X
    for _ in range(NS_ITERS):
        A = X @ X.T
        X = a * X + (b * A + c * (A @ A)) @ X
    return X.T if tall else X


def kernel(x, norm1_w, norm2_w, qkv_w, q_norm_w, k_norm_w, gamma_w1, gamma_w2,
           mem_wk, mem_w1, mem_w2, memory_gate, wo_w, ffn_w1, ffn_w2, ffn_w3):
    x = np.asarray(x, np.float32)
    f32 = np.float32

    h = _rmsnorm(x, np.asarray(norm1_w, f32))
    qkv = h.reshape(TOK, D) @ np.asarray(qkv_w, f32)
    q, k, v = np.split(qkv.reshape(B, S, 3 * D), 3, axis=-1)

    def heads(t):
        return t.reshape(B, S, H, HD).transpose(0, 2, 1, 3)

    q, k, v = heads(q), heads(k), heads(v)
    q = _rmsnorm(q, np.asarray(q_norm_w, f32))
    k = _rmsnorm(k, np.asarray(k_norm_w, f32))
    q, k = _rope(q, k)

    gamma = _sigmoid(_silu(h @ np.asarray(gamma_w1, f32)) @ np.asarray(gamma_w2, f32))

    k_cummean = np.cumsum(k, axis=2) / np.arange(1, S + 1, dtype=f32)[None, None, :, None]
    g = gamma[:, None, :, :]
    q_mem = g * q + (1.0 - g) * k_cummean
    q_mem_flat = q_mem.transpose(0, 2, 1, 3).reshape(B, S, D)
    v_flat = v.transpose(0, 2, 1, 3).reshape(B, S, D)

    # ---- TTL grads (w_omega nonzero only on last OMEGA_W positions) ----
    mem_wk = np.asarray(mem_wk, f32)
    mem_w1 = np.asarray(mem_w1, f32)
    mem_w2 = np.asarray(mem_w2, f32)
    pos = np.arange(S)
    dpow = (np.float32(OMEGA_DECAY) ** (S - 1 - pos).astype(f32)).astype(f32)
    dpow = np.where(pos >= S - OMEGA_W, dpow, 0.0).astype(f32)
    w_omega = gamma[..., 0] * dpow                     # (B,S)
    denom = np.sum(w_omega) + 1e-8

    T0 = S - OMEGA_W
    qm_t = q_mem_flat[:, T0:]                          # (B,64,D)
    v_t = v_flat[:, T0:]
    z_t = qm_t @ mem_wk                                # (B,64,HD)
    ft = _phi2(z_t)                                    # (B,64,F_POLY)
    a1 = ft @ mem_w1
    sg = _sigmoid(a1)
    h1 = a1 * sg
    pred = h1 @ mem_w2
    diff = pred - v_t
    dpred = (2.0 / denom) * w_omega[:, T0:, None] * diff
    g2 = np.einsum('btm,btd->md', h1, dpred).astype(f32)
    da1 = (dpred @ mem_w2.T) * (sg * (1.0 + a1 * (1.0 - sg)))
    g1 = np.einsum('btf,btm->fm', ft, da1).astype(f32)
    dfeat = da1 @ mem_w1.T
    dz = dfeat[..., :HD].copy()
    dO = dfeat[..., HD:].reshape(B, OMEGA_W, HD, HD)
    dz += np.einsum('btij,btj->bti', dO + dO.transpose(0, 1, 3, 2),
                    z_t) / np.sqrt(np.float32(HD))
    gk = np.einsum('btd,bte->de', qm_t, dz).astype(f32)

    mem_wk_u = TTL_ALPHA * mem_wk - TTL_ETA * _newton_schulz(gk)
    mem_w1_u = TTL_ALPHA * mem_w1 - TTL_ETA * _newton_schulz(g1)
    mem_w2_u = TTL_ALPHA * mem_w2 - TTL_ETA * _newton_schulz(g2)

    # ---- attention on host ----
    scale = HD ** -0.5
    attn_out = np.empty((B, H, S, HD), f32)
    causal_bias = np.triu(np.full((S, S), -np.inf, f32), 1)
    for b in range(B):
        for hh in range(H):
            sc = (q[b, hh] @ k[b, hh].T) * scale + causal_bias
            sc -= sc.max(-1, keepdims=True)
            e = np.exp(sc)
            p = e / e.sum(-1, keepdims=True)
            attn_out[b, hh] = p @ v[b, hh]
    attn_out = attn_out.transpose(0, 2, 1, 3).reshape(B, S, D) @ np.asarray(wo_w, f32)

    # ---- device: mem MLP fwd + residuals + rmsnorm + FFN ----
    r = (x + attn_out).reshape(TOK, D).astype(f32)
    z_full = (q_mem_flat @ mem_wk_u).reshape(TOK, HD)
    feat_f = _fold_feat(z_full)
    w1fold = _fold_w1(mem_w1_u)
    w2ms = (mem_w2_u * _sigmoid(np.asarray(memory_gate, f32))).astype(f32)

    # fold norm2_w into the FFN input weights: w.T @ (xmid*inv*n2w) =
    # inv * ((w*n2w).T @ xmid), with inv applied at psum eviction on device
    n2 = np.asarray(norm2_w, f32)[:, None]
    w1p = np.zeros((D, FFN_HP), f32)
    w1p[:, :FFN_H] = np.asarray(ffn_w1, f32) * n2
    w3p = np.zeros((D, FFN_HP), f32)
    w3p[:, :FFN_H] = np.asarray(ffn_w3, f32) * n2
    w2p = np.zeros((FFN_HP, D), f32)
    w2p[:FFN_H, :] = np.asarray(ffn_w2, f32)

    out = _run_device(feat_f, r, w1fold, w2ms, w1p, w3p, w2p,
                      np.asarray(norm2_w, f32))
    return out.reshape(B, S, D).astype(np.float32)

